# revision 5
# baseline (speedup 1.0000x reference)
import time

import numpy as np

NCORES = 8
H = W = 64
CIN = 2048
COUT = 512
PRE_NMS_TOPK = 2000
NMS_THR = 0.5

_NC = None
LAST_RES = None


def _rne11(x):
    # round-to-nearest-even to 11 explicit mantissa bits (TRN2 fp32r PE rounding)
    b = np.ascontiguousarray(x).view(np.uint32).astype(np.uint64)
    half = np.uint64((1 << 11) - 1)
    lsb = (b >> np.uint64(12)) & np.uint64(1)
    r = ((b + half + lsb) >> np.uint64(12)) << np.uint64(12)
    return r.astype(np.uint32).view(np.float32)


def _build():
    import concourse.bacc as bacc
    import concourse.mybir as mybir
    import concourse.tile as tile

    F32 = mybir.dt.float32
    F32R = mybir.dt.float32r
    AF = mybir.ActivationFunctionType

    nc = bacc.Bacc("TRN2", target_bir_lowering=False, debug=False, num_devices=NCORES)
    xr_d = nc.dram_tensor("xr", [8, 128, 66, 66], F32R, kind="ExternalInput")
    xl_d = nc.dram_tensor("xl", [8, 128, 66, 66], F32R, kind="ExternalInput")
    wr_d = nc.dram_tensor("wr", [8, 128, 9, 128], F32R, kind="ExternalInput")
    wl_d = nc.dram_tensor("wl", [8, 128, 9, 128], F32R, kind="ExternalInput")
    hwr_d = nc.dram_tensor("hwr", [128, 54], F32R, kind="ExternalInput")
    hwl_d = nc.dram_tensor("hwl", [128, 54], F32R, kind="ExternalInput")
    cb_d = nc.dram_tensor("cb", [128, 1], F32, kind="ExternalInput")
    po_d = nc.dram_tensor("po", [54, 8, 8, 64], F32, kind="ExternalOutput")

    with tile.TileContext(nc) as tc:
        with (
            tc.tile_pool(name="persist", bufs=1) as persist,
            tc.tile_pool(name="dram", bufs=1, space="DRAM") as dram,
        ):
            hwr_s = persist.tile([128, 54], F32R)
            hwl_s = persist.tile([128, 54], F32R)
            cb_s = persist.tile([128, 1], F32)
            nc.gpsimd.dma_start(hwr_s[:], hwr_d[:])
            nc.gpsimd.dma_start(hwl_s[:], hwl_d[:])
            nc.gpsimd.dma_start(cb_s[:], cb_d[:])
            conv_part = persist.tile([128, 8, 8, 64], F32)

            qs = [nc.sync, nc.scalar, nc.gpsimd]
            with (
                tc.tile_pool(name="psc", bufs=1, space="PSUM") as psc,
                tc.tile_pool(name="xrp", bufs=2) as xrp,
                tc.tile_pool(name="xlp", bufs=2) as xlp,
                tc.tile_pool(name="wrp", bufs=3) as wrp,
                tc.tile_pool(name="wlp", bufs=3) as wlp,
            ):
                ps = [psc.tile([128, 8, 64], F32, name=f"ps{n}") for n in range(8)]
                for kt in range(8):
                    xr_t = xrp.tile([128, 66, 66], F32R)
                    xl_t = xlp.tile([128, 66, 66], F32R)
                    wr_t = wrp.tile([128, 9, 128], F32R)
                    wl_t = wlp.tile([128, 9, 128], F32R)
                    qs[kt % 3].dma_start(xr_t[:], xr_d[kt])
                    qs[(kt + 1) % 3].dma_start(xl_t[:], xl_d[kt])
                    qs[(kt + 2) % 3].dma_start(wr_t[:], wr_d[kt])
                    qs[(kt + 2) % 3].dma_start(wl_t[:], wl_d[kt])
                    for tap in range(9):
                        ky, kx = divmod(tap, 3)
                        for p, (wt, xt) in enumerate(
                            [(wr_t, xr_t), (wl_t, xr_t), (wr_t, xl_t)]
                        ):
                            for nch in range(8):
                                nc.tensor.matmul(
                                    ps[nch][:, :, :],
                                    wt[:, tap, :],
                                    xt[:, nch * 8 + ky : nch * 8 + ky + 8, kx : kx + 64],
                                    start=(kt == 0 and tap == 0 and p == 0),
                                    stop=(kt == 7 and tap == 8 and p == 2),
                                )
                for nch in range(8):
                    nc.vector.tensor_copy(conv_part[:, nch], ps[nch][:])

            arin_b = dram.tile([128, 8, 8, 64], F32)
            arout_b = dram.tile([128, 8, 8, 64], F32)
            nc.gpsimd.dma_start(arin_b[:], conv_part[:])
            nc.gpsimd.collective_compute(
                "AllReduce",
                mybir.AluOpType.add,
                replica_groups=[[0, 1], [2, 3], [4, 5], [6, 7]],
                ins=[arin_b.opt()],
                outs=[arout_b.opt()],
            )
            conv_full = persist.tile([128, 8, 8, 64], F32)
            nc.gpsimd.dma_start(conv_full[:], arout_b[:])

            feat = persist.tile([128, 8, 8, 64], F32)
            nc.scalar.activation(feat[:], conv_full[:], AF.Relu, bias=cb_s[:], scale=1.0)
            feat_r = persist.tile([128, 8, 8, 64], F32R)
            nc.vector.tensor_copy(feat_r[:], feat[:])
            feat_lo = persist.tile([128, 8, 8, 64], F32)
            nc.vector.tensor_sub(feat_lo[:], feat[:], feat_r[:].bitcast(F32))
            feat_lo_r = persist.tile([128, 8, 8, 64], F32R)
            nc.vector.tensor_copy(feat_lo_r[:], feat_lo[:])

            po_s = persist.tile([54, 8, 8, 64], F32)
            with tc.tile_pool(name="psh", bufs=2, space="PSUM") as psh:
                for ch in range(8):
                    ph = psh.tile([54, 8, 64], F32)
                    for p, (ht, ft) in enumerate(
                        [(hwr_s, feat_r), (hwl_s, feat_r), (hwr_s, feat_lo_r)]
                    ):
                        nc.tensor.matmul(
                            ph[:], ht[:], ft[:, ch], start=(p == 0), stop=(p == 2)
                        )
                    nc.vector.tensor_copy(po_s[:, ch], ph[:])
            nc.sync.dma_start(po_d[:], po_s[:])

    nc.compile()
    return nc


def _get_nc():
    global _NC
    if _NC is None:
        _NC = _build()
    return _NC


def _gen_anchors():
    scales = np.array([8.0, 16.0, 32.0], np.float32)
    ratios = np.array([0.5, 1.0, 2.0], np.float32)
    ws = (scales[:, None] * np.sqrt(ratios)[None, :]).reshape(-1)
    hs = (scales[:, None] / np.sqrt(ratios)[None, :]).reshape(-1)
    base = np.stack([-ws / 2, -hs / 2, ws / 2, hs / 2], axis=-1).astype(np.float32)
    sy, sx = np.meshgrid(np.arange(H) * 32, np.arange(W) * 32, indexing="ij")
    shifts = np.stack([sx, sy, sx, sy], axis=-1).reshape(-1, 4).astype(np.float32)
    return (base[:, None, :] + shifts[None, :, :]).reshape(-1, 4)


def kernel(**inputs):
    global LAST_RES
    from concourse.bass_utils import run_bass_kernel_spmd

    x = np.ascontiguousarray(np.asarray(inputs["x"], np.float32))
    conv_w = np.asarray(inputs["conv_w"], np.float32)
    conv_b = np.asarray(inputs["conv_b"], np.float32)
    cls_w = np.asarray(inputs["cls_w"], np.float32)
    cls_b = np.asarray(inputs["cls_b"], np.float32)
    bbox_w = np.asarray(inputs["bbox_w"], np.float32)
    bbox_b = np.asarray(inputs["bbox_b"], np.float32)

    nc = _get_nc()

    xp = np.zeros((CIN, 66, 66), np.float32)
    xp[:, 1:65, 1:65] = x[0]
    xlo = xp - _rne11(xp)

    in_maps = []
    for c in range(NCORES):
        i, j = divmod(c, 2)
        xg = xp[j * 1024 : (j + 1) * 1024].reshape(8, 128, 66, 66)
        xg_lo = xlo[j * 1024 : (j + 1) * 1024].reshape(8, 128, 66, 66)
        wfull = conv_w[i * 128 : (i + 1) * 128, j * 1024 : (j + 1) * 1024]
        wt = np.ascontiguousarray(wfull.transpose(1, 2, 3, 0)).reshape(8, 128, 9, 128)
        wlo = wt - _rne11(wt)
        hw = np.ascontiguousarray(
            np.concatenate(
                [cls_w[:, i * 128 : (i + 1) * 128], bbox_w[:, i * 128 : (i + 1) * 128]],
                axis=0,
            ).T
        )
        hwlo = hw - _rne11(hw)
        cb = np.ascontiguousarray(conv_b[i * 128 : (i + 1) * 128].reshape(128, 1))
        in_maps.append(
            {"xr": xg, "xl": xg_lo, "wr": wt, "wl": wlo, "hwr": hw, "hwl": hwlo, "cb": cb}
        )

    t0 = time.time()
    res = run_bass_kernel_spmd(nc, in_maps, list(range(NCORES)))
    LAST_RES = res
    kernel.last_run_wall = time.time() - t0

    po = np.zeros((54, 4096), np.float64)
    for i in range(4):
        po += res.results[2 * i]["po"].reshape(54, 4096).astype(np.float64)

    obj = po[:18].astype(np.float32) + cls_b[:, None]
    deltas = po[18:].astype(np.float32) + bbox_b[:, None]

    o = obj.reshape(2, 9, H, W)
    m = np.maximum(o[0], o[1])
    e0 = np.exp(o[0] - m)
    e1 = np.exp(o[1] - m)
    probs = (e1 / (e0 + e1)).reshape(-1)

    d = deltas.reshape(9, 4, H, W).transpose(0, 2, 3, 1).reshape(-1, 4)
    anchors = _gen_anchors()
    aw = anchors[:, 2] - anchors[:, 0]
    ah = anchors[:, 3] - anchors[:, 1]
    acx = anchors[:, 0] + 0.5 * aw
    acy = anchors[:, 1] + 0.5 * ah
    pcx = acx + d[:, 0] * aw
    pcy = acy + d[:, 1] * ah
    pw = np.exp(d[:, 2]) * aw
    ph = np.exp(d[:, 3]) * ah
    boxes = np.stack(
        [pcx - 0.5 * pw, pcy - 0.5 * ph, pcx + 0.5 * pw, pcy + 0.5 * ph], axis=-1
    )

    order = np.argsort(-probs, kind="stable")[:PRE_NMS_TOPK]
    top_scores = probs[order]
    top_boxes = boxes[order]

    b = top_boxes
    area = (b[:, 2] - b[:, 0]) * (b[:, 3] - b[:, 1])
    lt = np.maximum(b[:, None, :2], b[None, :, :2])
    rb = np.minimum(b[:, None, 2:], b[None, :, 2:])
    wh = np.clip(rb - lt, 0.0, None)
    inter = wh[..., 0] * wh[..., 1]
    iou = inter / (area[:, None] + area[None, :] - inter + np.float32(1e-9))

    K = PRE_NMS_TOPK
    keep = np.ones(K, bool)
    idx = np.arange(K)
    for i_ in range(K):
        if keep[i_]:
            keep &= ~((iou[i_] > NMS_THR) & (idx > i_))
    kf = keep.astype(np.float32)
    return top_boxes * kf[:, None], top_scores * kf
